# revision 45
# baseline (speedup 1.0000x reference)
"""Bahdanau-attention kernel for Trainium2 (8 NeuronCores, SPMD data parallel).

Math: the reference's per-step softmax is over a singleton axis, so the
attention weights are exactly 1.0. Hence:
    context  = values.sum(axis=1)            [B, DV]
    attn     = ones(B, T, 1)
    coverage[b, t, 0] = t                    [B, T, 1]
The W1/W2/W3/V MLP cancels out of every output.

Device work: per core, reduce a [B/8, T, DV] shard of `values` over T.
All 64 chunk loads stream on the sync HWDGE ring (outputs go on the
scalar ring so they never head-of-line block the loads); the fp32 adds
are spread over three engines so none exceeds the HBM/DMA roofline:
  - DVE: two short tensor_add chains per batch (dacc_a, dacc_b)
  - GpSimd: 4 chunks per batch in its own chain (gacc), folded into
    dacc_a mid-stream
  - PE: contracts dacc_a/dacc_b over partitions vs a ones column into
    the per-batch PSUM accumulation group.
attn/coverage come from a tiny host const tensor, written out by DMA.
"""

import os
import numpy as np

B, T, DV = 32, 2048, 1024
NCORES = 8
BP = B // NCORES          # 4 batches per core
TCH = 128                 # t-chunk rows = SBUF partitions
NCH = T // TCH            # 16 chunks of [128, DV] per batch
NSPLIT = 512              # PSUM bank free-dim limit (f32)
NJ = DV // NSPLIT         # 2 psum column groups

# Engine roles (measured per-chunk costs: DVE add 1.22us, gpsimd add
# ~2.4us, PE direct contraction ~2.5us = 4 MM insts; gpsimd and DVE adds
# contend on SBUF ports, so gpsimd's share is kept small):
#   - gpsimd: 4 chunks per batch in its own chain (gacc)
#   - DVE: two short chains per batch (dacc_a over A_CHUNKS, dacc_b over
#     B_CHUNKS); gacc folds into dacc_a after chunk GP_FOLD_AFTER
#   - PE: only the two cross-partition contractions per batch
GP_CHUNKS = (1, 4, 7, 10)
A_CHUNKS = (0, 2, 3, 5, 6)
B_CHUNKS = (8, 9, 11, 12, 13, 14, 15)
PE_CHUNK = None
PE_CHUNK_LAST_B = 15      # last batch: chunk 15 contracts directly on PE
GP_FOLD_AFTER = 13        # gacc -> dacc_a fold point (gacc done by ~k13)
CROSSFADE = 6             # loads interleaved across each batch boundary


def _load_schedule():
    """Load order: batch-major, but each boundary crossfades the last
    CROSSFADE chunks of b with the first CROSSFADE of b+1, so the batch-end
    reduction convoy (gpsimd tail + fold + contraction) drains while fresh
    chunks with idle consumers keep freeing buffer slots."""
    order = []
    for b in range(BP):
        start = CROSSFADE if b > 0 else 0
        end = NCH - CROSSFADE if b < BP - 1 else NCH
        order.extend((b, k) for k in range(start, end))
        if b < BP - 1:
            for i in range(CROSSFADE):
                order.append((b, NCH - CROSSFADE + i))
                order.append((b + 1, i))
    assert sorted(order) == [(b, k) for b in range(BP) for k in range(NCH)]
    return order

_CACHE = {}
LAST = {}                 # exec_time_ns etc. for the test harness


def _build_nc():
    import concourse.tile as tile
    from concourse import bacc, mybir
    from contextlib import ExitStack

    f32 = mybir.dt.float32
    nc = bacc.Bacc(
        "TRN2", target_bir_lowering=False, debug=False, num_devices=NCORES
    )

    vals = nc.dram_tensor("vals", [BP, T, DV], f32, kind="ExternalInput").ap()
    consts = nc.dram_tensor("consts", [2, T], f32, kind="ExternalInput").ap()
    ctx_out = nc.dram_tensor("ctx_out", [BP, DV], f32, kind="ExternalOutput").ap()
    attn_out = nc.dram_tensor("attn_out", [BP, T, 1], f32, kind="ExternalOutput").ap()
    cov_out = nc.dram_tensor("cov_out", [BP, T, 1], f32, kind="ExternalOutput").ap()

    with tile.TileContext(nc) as tc, ExitStack() as ctx:
        cpool = ctx.enter_context(tc.tile_pool(name="const", bufs=1))
        vpool = ctx.enter_context(tc.tile_pool(name="vals", bufs=24))
        dpool = ctx.enter_context(tc.tile_pool(name="dacc", bufs=1))
        ppool = ctx.enter_context(tc.tile_pool(name="ps", bufs=1, space="PSUM"))
        opool = ctx.enter_context(tc.tile_pool(name="out", bufs=2))

        ones_t = cpool.tile([128, 1], f32)
        nc.vector.memset(ones_t[:], 1.0)

        const_t = cpool.tile([2, T], f32)
        nc.sync.dma_start(out=const_t[:], in_=consts[:])

        # attn/coverage writes go on the scalar HWDGE ring, issued as soon
        # as const_t lands, so the sync ring carries only the big loads.
        for b in range(BP):
            nc.scalar.dma_start(out=attn_out[b:b + 1, :, 0], in_=const_t[0:1, :])
            nc.scalar.dma_start(out=cov_out[b:b + 1, :, 0], in_=const_t[1:2, :])

        st = {}
        for b in range(BP):
            st[b] = {
                "dacc_a": dpool.tile(
                    [TCH, DV], f32, name=f"dacca{b}", tag=f"dacca{b}"),
                "dacc_b": dpool.tile(
                    [TCH, DV], f32, name=f"daccb{b}", tag=f"daccb{b}"),
                "gacc": dpool.tile(
                    [TCH, DV], f32, name=f"gacc{b}", tag=f"gacc{b}"),
                "na": 0, "nb": 0, "ngp": 0,
                "afirst": None, "bfirst": None, "gfirst": None,
                "pe_tile": None,
            }

        for b, k in _load_schedule():
            s = st[b]
            vt = vpool.tile([TCH, DV], f32, name=f"vt{b}_{k}", tag="vt")
            nc.sync.dma_start(
                out=vt[:], in_=vals[b, k * TCH:(k + 1) * TCH, :])
            if k == PE_CHUNK or (b == BP - 1 and k == PE_CHUNK_LAST_B):
                s["pe_tile"] = vt
            elif k in GP_CHUNKS:
                s["ngp"] += 1
                if s["ngp"] == 1:
                    s["gfirst"] = vt
                elif s["ngp"] == 2:
                    nc.gpsimd.tensor_add(s["gacc"][:], s["gfirst"][:], vt[:])
                else:
                    nc.gpsimd.tensor_add(s["gacc"][:], s["gacc"][:], vt[:])
            elif k in A_CHUNKS:
                s["na"] += 1
                if s["na"] == 1:
                    s["afirst"] = vt
                elif s["na"] == 2:
                    nc.vector.tensor_add(s["dacc_a"][:], s["afirst"][:], vt[:])
                else:
                    nc.vector.tensor_add(s["dacc_a"][:], s["dacc_a"][:], vt[:])
            else:
                s["nb"] += 1
                if s["nb"] == 1:
                    s["bfirst"] = vt
                elif s["nb"] == 2:
                    nc.vector.tensor_add(s["dacc_b"][:], s["bfirst"][:], vt[:])
                else:
                    nc.vector.tensor_add(s["dacc_b"][:], s["dacc_b"][:], vt[:])
            if k == GP_FOLD_AFTER:
                nc.vector.tensor_add(s["dacc_a"][:], s["dacc_a"][:], s["gacc"][:])

            if k != NCH - 1:
                continue
            # b's final chunk scheduled: contract and write out
            ps = [
                ppool.tile([1, NSPLIT], f32, name=f"ps{b}_{j}", tag=f"ps{b}_{j}")
                for j in range(NJ)
            ]
            srcs = [s["dacc_a"], s["dacc_b"]]
            if s["pe_tile"] is not None:
                srcs.append(s["pe_tile"])
            for i, src in enumerate(srcs):
                for j in range(NJ):
                    sl = slice(j * NSPLIT, (j + 1) * NSPLIT)
                    nc.tensor.matmul(
                        ps[j][:], ones_t[:], src[:, sl],
                        start=(i == 0), stop=(i == len(srcs) - 1))

            ot = opool.tile([1, DV], f32, name=f"ot{b}", tag="ot")
            for j in range(NJ):
                nc.scalar.copy(ot[:, j * NSPLIT:(j + 1) * NSPLIT], ps[j][:])
            # context write on the scalar ring: keeps the sync HWDGE FIFO
            # free of output DMAs that would head-of-line block later loads
            nc.scalar.dma_start(out=ctx_out[b:b + 1, :], in_=ot[0:1, :])

    nc.compile()
    return nc


def kernel(query=None, values=None, **unused_weights):
    from concourse.bass_utils import run_bass_kernel_spmd

    values = np.ascontiguousarray(np.asarray(values, dtype=np.float32))
    assert values.shape == (B, T, DV), values.shape

    if "nc" not in _CACHE:
        _CACHE["nc"] = _build_nc()
    nc = _CACHE["nc"]

    consts = np.stack(
        [np.ones(T, dtype=np.float32), np.arange(T, dtype=np.float32)]
    )
    core_ids = list(range(NCORES))
    in_maps = [
        {"vals": values[c * BP:(c + 1) * BP], "consts": consts}
        for c in core_ids
    ]

    trace = bool(int(os.environ.get("BASS_KERNEL_TRACE", "0")))
    res = run_bass_kernel_spmd(nc, in_maps, core_ids, trace=trace)
    LAST["exec_time_ns"] = res.exec_time_ns
    LAST["results"] = res

    context = np.concatenate([res.results[c]["ctx_out"] for c in core_ids], axis=0)
    attn = np.concatenate([res.results[c]["attn_out"] for c in core_ids], axis=0)
    coverage = np.concatenate([res.results[c]["cov_out"] for c in core_ids], axis=0)
    return context, attn, coverage


# revision 46
# speedup vs baseline: 1.0239x; 1.0239x over previous
"""Bahdanau-attention kernel for Trainium2 (8 NeuronCores, SPMD data parallel).

Math: the reference's per-step softmax is over a singleton axis, so the
attention weights are exactly 1.0. Hence:
    context  = values.sum(axis=1)            [B, DV]
    attn     = ones(B, T, 1)
    coverage[b, t, 0] = t                    [B, T, 1]
The W1/W2/W3/V MLP cancels out of every output.

Device work: per core, reduce a [B/8, T, DV] shard of `values` over T.
All 64 chunk loads stream on the sync HWDGE ring (outputs go on the
scalar ring so they never head-of-line block the loads); the fp32 adds
are spread over three engines so none exceeds the HBM/DMA roofline:
  - DVE: two short tensor_add chains per batch (dacc_a, dacc_b)
  - GpSimd: 4 chunks per batch in its own chain (gacc), folded into
    dacc_a mid-stream
  - PE: contracts dacc_a/dacc_b over partitions vs a ones column into
    the per-batch PSUM accumulation group.
attn/coverage come from a tiny host const tensor, written out by DMA.
"""

import os
import numpy as np

B, T, DV = 32, 2048, 1024
NCORES = 8
BP = B // NCORES          # 4 batches per core
TCH = 128                 # t-chunk rows = SBUF partitions
NCH = T // TCH            # 16 chunks of [128, DV] per batch
NSPLIT = 512              # PSUM bank free-dim limit (f32)
NJ = DV // NSPLIT         # 2 psum column groups

# Engine roles (measured per-chunk costs: DVE add 1.22us, gpsimd add
# ~2.4us, PE direct contraction ~2.5us = 4 MM insts; gpsimd and DVE adds
# contend on SBUF ports, so gpsimd's share is kept small):
#   - gpsimd: 4 chunks per batch in its own chain (gacc)
#   - DVE: two short chains per batch (dacc_a over A_CHUNKS, dacc_b over
#     B_CHUNKS); gacc folds into dacc_a after chunk GP_FOLD_AFTER
#   - PE: only the two cross-partition contractions per batch
GP_CHUNKS = (1, 4, 7, 10)
A_CHUNKS = (0, 2, 3, 5, 6)
B_CHUNKS = (8, 9, 11, 12, 13, 14, 15)
PE_CHUNK = None
PE_CHUNK_LAST_B = None    # optional: last batch's chunk 15 direct on PE
GP_FOLD_AFTER = 13        # gacc -> dacc_a fold point (gacc done by ~k13)
CROSSFADE = 4             # loads interleaved across each batch boundary


def _load_schedule():
    """Load order: batch-major, but each boundary crossfades the last
    CROSSFADE chunks of b with the first CROSSFADE of b+1, so the batch-end
    reduction convoy (gpsimd tail + fold + contraction) drains while fresh
    chunks with idle consumers keep freeing buffer slots."""
    order = []
    for b in range(BP):
        start = CROSSFADE if b > 0 else 0
        end = NCH - CROSSFADE if b < BP - 1 else NCH
        order.extend((b, k) for k in range(start, end))
        if b < BP - 1:
            for i in range(CROSSFADE):
                order.append((b, NCH - CROSSFADE + i))
                order.append((b + 1, i))
    assert sorted(order) == [(b, k) for b in range(BP) for k in range(NCH)]
    return order

_CACHE = {}
LAST = {}                 # exec_time_ns etc. for the test harness


def _build_nc():
    import concourse.tile as tile
    from concourse import bacc, mybir
    from contextlib import ExitStack

    f32 = mybir.dt.float32
    nc = bacc.Bacc(
        "TRN2", target_bir_lowering=False, debug=False, num_devices=NCORES
    )

    vals = nc.dram_tensor("vals", [BP, T, DV], f32, kind="ExternalInput").ap()
    consts = nc.dram_tensor("consts", [2, T], f32, kind="ExternalInput").ap()
    ctx_out = nc.dram_tensor("ctx_out", [BP, DV], f32, kind="ExternalOutput").ap()
    attn_out = nc.dram_tensor("attn_out", [BP, T, 1], f32, kind="ExternalOutput").ap()
    cov_out = nc.dram_tensor("cov_out", [BP, T, 1], f32, kind="ExternalOutput").ap()

    with tile.TileContext(nc) as tc, ExitStack() as ctx:
        cpool = ctx.enter_context(tc.tile_pool(name="const", bufs=1))
        vpool = ctx.enter_context(tc.tile_pool(name="vals", bufs=24))
        dpool = ctx.enter_context(tc.tile_pool(name="dacc", bufs=1))
        ppool = ctx.enter_context(tc.tile_pool(name="ps", bufs=1, space="PSUM"))
        opool = ctx.enter_context(tc.tile_pool(name="out", bufs=2))

        ones_t = cpool.tile([128, 1], f32)
        nc.vector.memset(ones_t[:], 1.0)

        const_t = cpool.tile([2, T], f32)
        nc.sync.dma_start(out=const_t[:], in_=consts[:])

        # attn/coverage writes go on the scalar HWDGE ring, issued as soon
        # as const_t lands, so the sync ring carries only the big loads.
        for b in range(BP):
            nc.scalar.dma_start(out=attn_out[b:b + 1, :, 0], in_=const_t[0:1, :])
            nc.scalar.dma_start(out=cov_out[b:b + 1, :, 0], in_=const_t[1:2, :])

        st = {}
        for b in range(BP):
            st[b] = {
                "dacc_a": dpool.tile(
                    [TCH, DV], f32, name=f"dacca{b}", tag=f"dacca{b}"),
                "dacc_b": dpool.tile(
                    [TCH, DV], f32, name=f"daccb{b}", tag=f"daccb{b}"),
                "gacc": dpool.tile(
                    [TCH, DV], f32, name=f"gacc{b}", tag=f"gacc{b}"),
                "na": 0, "nb": 0, "ngp": 0,
                "afirst": None, "bfirst": None, "gfirst": None,
                "pe_tile": None,
            }

        for b, k in _load_schedule():
            s = st[b]
            vt = vpool.tile([TCH, DV], f32, name=f"vt{b}_{k}", tag="vt")
            nc.sync.dma_start(
                out=vt[:], in_=vals[b, k * TCH:(k + 1) * TCH, :])
            if k == PE_CHUNK or (b == BP - 1 and k == PE_CHUNK_LAST_B):
                s["pe_tile"] = vt
            elif k in GP_CHUNKS:
                s["ngp"] += 1
                if s["ngp"] == 1:
                    s["gfirst"] = vt
                elif s["ngp"] == 2:
                    nc.gpsimd.tensor_add(s["gacc"][:], s["gfirst"][:], vt[:])
                else:
                    nc.gpsimd.tensor_add(s["gacc"][:], s["gacc"][:], vt[:])
            elif k in A_CHUNKS:
                s["na"] += 1
                if s["na"] == 1:
                    s["afirst"] = vt
                elif s["na"] == 2:
                    nc.vector.tensor_add(s["dacc_a"][:], s["afirst"][:], vt[:])
                else:
                    nc.vector.tensor_add(s["dacc_a"][:], s["dacc_a"][:], vt[:])
            else:
                s["nb"] += 1
                if s["nb"] == 1:
                    s["bfirst"] = vt
                elif s["nb"] == 2:
                    nc.vector.tensor_add(s["dacc_b"][:], s["bfirst"][:], vt[:])
                else:
                    nc.vector.tensor_add(s["dacc_b"][:], s["dacc_b"][:], vt[:])
            if k == GP_FOLD_AFTER:
                nc.vector.tensor_add(s["dacc_a"][:], s["dacc_a"][:], s["gacc"][:])

            if k != NCH - 1:
                continue
            # b's final chunk scheduled: contract and write out
            ps = [
                ppool.tile([1, NSPLIT], f32, name=f"ps{b}_{j}", tag=f"ps{b}_{j}")
                for j in range(NJ)
            ]
            srcs = [s["dacc_a"], s["dacc_b"]]
            if s["pe_tile"] is not None:
                srcs.append(s["pe_tile"])
            for i, src in enumerate(srcs):
                for j in range(NJ):
                    sl = slice(j * NSPLIT, (j + 1) * NSPLIT)
                    nc.tensor.matmul(
                        ps[j][:], ones_t[:], src[:, sl],
                        start=(i == 0), stop=(i == len(srcs) - 1))

            ot = opool.tile([1, DV], f32, name=f"ot{b}", tag="ot")
            for j in range(NJ):
                nc.scalar.copy(ot[:, j * NSPLIT:(j + 1) * NSPLIT], ps[j][:])
            # context write on the scalar ring: keeps the sync HWDGE FIFO
            # free of output DMAs that would head-of-line block later loads
            nc.scalar.dma_start(out=ctx_out[b:b + 1, :], in_=ot[0:1, :])

    nc.compile()
    return nc


def kernel(query=None, values=None, **unused_weights):
    from concourse.bass_utils import run_bass_kernel_spmd

    values = np.ascontiguousarray(np.asarray(values, dtype=np.float32))
    assert values.shape == (B, T, DV), values.shape

    if "nc" not in _CACHE:
        _CACHE["nc"] = _build_nc()
    nc = _CACHE["nc"]

    consts = np.stack(
        [np.ones(T, dtype=np.float32), np.arange(T, dtype=np.float32)]
    )
    core_ids = list(range(NCORES))
    in_maps = [
        {"vals": values[c * BP:(c + 1) * BP], "consts": consts}
        for c in core_ids
    ]

    trace = bool(int(os.environ.get("BASS_KERNEL_TRACE", "0")))
    res = run_bass_kernel_spmd(nc, in_maps, core_ids, trace=trace)
    LAST["exec_time_ns"] = res.exec_time_ns
    LAST["results"] = res

    context = np.concatenate([res.results[c]["ctx_out"] for c in core_ids], axis=0)
    attn = np.concatenate([res.results[c]["attn_out"] for c in core_ids], axis=0)
    coverage = np.concatenate([res.results[c]["cov_out"] for c in core_ids], axis=0)
    return context, attn, coverage


# revision 50
# speedup vs baseline: 1.1257x; 1.0994x over previous
"""Bahdanau-attention kernel for Trainium2 (8 NeuronCores, SPMD data parallel).

Math: the reference's per-step softmax is over a singleton axis, so the
attention weights are exactly 1.0. Hence:
    context  = values.sum(axis=1)            [B, DV]
    attn     = ones(B, T, 1)
    coverage[b, t, 0] = t                    [B, T, 1]
The W1/W2/W3/V MLP cancels out of every output.

Device work: per core, reduce a [B/8, T, DV] shard of `values` over T.
All 64 chunk loads stream on the sync HWDGE ring (outputs go on the
scalar ring so they never head-of-line block the loads); the fp32 adds
are spread over three engines so none exceeds the HBM/DMA roofline:
  - DVE: two short tensor_add chains per batch (dacc_a, dacc_b)
  - GpSimd: 4 chunks per batch in its own chain (gacc), folded into
    dacc_a mid-stream
  - PE: contracts dacc_a/dacc_b over partitions vs a ones column into
    the per-batch PSUM accumulation group.
attn/coverage come from a tiny host const tensor, written out by DMA.
"""

import os
import numpy as np

B, T, DV = 32, 2048, 1024
NCORES = 8
BP = B // NCORES          # 4 batches per core
TCH = 128                 # t-chunk rows = SBUF partitions
NCH = T // TCH            # 16 chunks of [128, DV] per batch
NSPLIT = 512              # PSUM bank free-dim limit (f32)
NJ = DV // NSPLIT         # 2 psum column groups

# Engine roles (measured per-chunk costs: DVE add 1.22us, gpsimd add
# ~2.4us, PE direct contraction ~2.5us = 4 MM insts; gpsimd and DVE adds
# contend on SBUF ports, so gpsimd's share is kept small):
#   - gpsimd: 4 chunks per batch in its own chain (gacc)
#   - DVE: two short chains per batch (dacc_a over A_CHUNKS, dacc_b over
#     B_CHUNKS); gacc folds into dacc_a after chunk GP_FOLD_AFTER
#   - PE: only the two cross-partition contractions per batch
GP_CHUNKS = (1, 4, 7, 10)
A_CHUNKS = (0, 2, 3, 5, 6)
B_CHUNKS = (8, 9, 11, 12, 13, 14, 15)
PE_CHUNK = None
PE_CHUNK_LAST_B = None    # optional: last batch's chunk 15 direct on PE
GP_FOLD_AFTER = 13        # gacc -> dacc_a fold point (gacc done by ~k13)
PAIR_LOADS = True         # load chunk pairs as single 1MB DMAs
NPAIR = NCH // 2          # 8 pair-loads per batch
CROSSFADE = 2             # pair-loads interleaved across each batch boundary


def _load_schedule():
    """Pair-load order: batch-major, but each boundary crossfades the last
    CROSSFADE pair-loads of b with the first CROSSFADE of b+1, so the
    batch-end reduction convoy (gpsimd tail + fold + contraction) drains
    while fresh chunks with idle consumers keep freeing buffer slots."""
    order = []
    for b in range(BP):
        start = CROSSFADE if b > 0 else 0
        end = NPAIR - CROSSFADE if b < BP - 1 else NPAIR
        order.extend((b, p) for p in range(start, end))
        if b < BP - 1:
            for i in range(CROSSFADE):
                order.append((b, NPAIR - CROSSFADE + i))
                order.append((b + 1, i))
    assert sorted(order) == [(b, p) for b in range(BP) for p in range(NPAIR)]
    return order

_CACHE = {}
LAST = {}                 # exec_time_ns etc. for the test harness


def _build_nc():
    import concourse.tile as tile
    from concourse import bacc, mybir
    from contextlib import ExitStack

    f32 = mybir.dt.float32
    nc = bacc.Bacc(
        "TRN2", target_bir_lowering=False, debug=False, num_devices=NCORES
    )

    vals = nc.dram_tensor("vals", [BP, T, DV], f32, kind="ExternalInput").ap()
    consts = nc.dram_tensor("consts", [2, T], f32, kind="ExternalInput").ap()
    ctx_out = nc.dram_tensor("ctx_out", [BP, DV], f32, kind="ExternalOutput").ap()
    attn_out = nc.dram_tensor("attn_out", [BP, T, 1], f32, kind="ExternalOutput").ap()
    cov_out = nc.dram_tensor("cov_out", [BP, T, 1], f32, kind="ExternalOutput").ap()

    with tile.TileContext(nc) as tc, ExitStack() as ctx:
        cpool = ctx.enter_context(tc.tile_pool(name="const", bufs=1))
        vpool = ctx.enter_context(tc.tile_pool(name="vals", bufs=12))
        dpool = ctx.enter_context(tc.tile_pool(name="dacc", bufs=1))
        ppool = ctx.enter_context(tc.tile_pool(name="ps", bufs=1, space="PSUM"))
        opool = ctx.enter_context(tc.tile_pool(name="out", bufs=2))

        ones_t = cpool.tile([128, 1], f32)
        nc.vector.memset(ones_t[:], 1.0)

        const_t = cpool.tile([2, T], f32)
        nc.sync.dma_start(out=const_t[:], in_=consts[:])

        # attn/coverage writes go on the scalar HWDGE ring, issued as soon
        # as const_t lands, so the sync ring carries only the big loads.
        for b in range(BP):
            nc.scalar.dma_start(out=attn_out[b:b + 1, :, 0], in_=const_t[0:1, :])
            nc.scalar.dma_start(out=cov_out[b:b + 1, :, 0], in_=const_t[1:2, :])

        st = {}
        for b in range(BP):
            st[b] = {
                "dacc_a": dpool.tile(
                    [TCH, DV], f32, name=f"dacca{b}", tag=f"dacca{b}"),
                "dacc_b": dpool.tile(
                    [TCH, DV], f32, name=f"daccb{b}", tag=f"daccb{b}"),
                "gacc": dpool.tile(
                    [TCH, DV], f32, name=f"gacc{b}", tag=f"gacc{b}"),
                "na": 0, "nb": 0, "ngp": 0,
                "afirst": None, "bfirst": None, "gfirst": None,
                "pe_tile": None,
            }

        def dispatch(s, b, k, vt):
            """Feed one [128, DV] chunk view to its assigned engine."""
            if k == PE_CHUNK or (b == BP - 1 and k == PE_CHUNK_LAST_B):
                s["pe_tile"] = vt
            elif k in GP_CHUNKS:
                s["ngp"] += 1
                if s["ngp"] == 1:
                    s["gfirst"] = vt
                elif s["ngp"] == 2:
                    nc.gpsimd.tensor_add(s["gacc"][:], s["gfirst"], vt)
                else:
                    nc.gpsimd.tensor_add(s["gacc"][:], s["gacc"][:], vt)
            elif k in A_CHUNKS:
                s["na"] += 1
                if s["na"] == 1:
                    s["afirst"] = vt
                elif s["na"] == 2:
                    nc.vector.tensor_add(s["dacc_a"][:], s["afirst"], vt)
                else:
                    nc.vector.tensor_add(s["dacc_a"][:], s["dacc_a"][:], vt)
            else:
                s["nb"] += 1
                if s["nb"] == 1:
                    s["bfirst"] = vt
                elif s["nb"] == 2:
                    nc.vector.tensor_add(s["dacc_b"][:], s["bfirst"], vt)
                else:
                    nc.vector.tensor_add(s["dacc_b"][:], s["dacc_b"][:], vt)
            if k == GP_FOLD_AFTER:
                nc.vector.tensor_add(s["dacc_a"][:], s["dacc_a"][:], s["gacc"][:])

        for b, p in _load_schedule():
            s = st[b]
            # one 1MB DMA covers chunks (2p, 2p+1): [p, g, d] view of vals
            vr = vals[b].rearrange("(g p) d -> p g d", p=TCH)
            pt = vpool.tile([TCH, 2, DV], f32, name=f"pt{b}_{p}", tag="pt")
            nc.sync.dma_start(out=pt[:], in_=vr[:, 2 * p:2 * p + 2, :])
            dispatch(s, b, 2 * p, pt[:, 0, :])
            dispatch(s, b, 2 * p + 1, pt[:, 1, :])

            if p != NPAIR - 1:
                continue
            # b's final chunk scheduled: contract and write out
            ps = [
                ppool.tile([1, NSPLIT], f32, name=f"ps{b}_{j}", tag=f"ps{b}_{j}")
                for j in range(NJ)
            ]
            srcs = [s["dacc_a"], s["dacc_b"]]
            if s["pe_tile"] is not None:
                srcs.append(s["pe_tile"])
            for i, src in enumerate(srcs):
                for j in range(NJ):
                    sl = slice(j * NSPLIT, (j + 1) * NSPLIT)
                    nc.tensor.matmul(
                        ps[j][:], ones_t[:], src[:, sl],
                        start=(i == 0), stop=(i == len(srcs) - 1))

            ot = opool.tile([1, DV], f32, name=f"ot{b}", tag="ot")
            for j in range(NJ):
                nc.scalar.copy(ot[:, j * NSPLIT:(j + 1) * NSPLIT], ps[j][:])
            # context write on the scalar ring: keeps the sync HWDGE FIFO
            # free of output DMAs that would head-of-line block later loads
            nc.scalar.dma_start(out=ctx_out[b:b + 1, :], in_=ot[0:1, :])

    nc.compile()
    return nc


def kernel(query=None, values=None, **unused_weights):
    from concourse.bass_utils import run_bass_kernel_spmd

    values = np.ascontiguousarray(np.asarray(values, dtype=np.float32))
    assert values.shape == (B, T, DV), values.shape

    if "nc" not in _CACHE:
        _CACHE["nc"] = _build_nc()
    nc = _CACHE["nc"]

    consts = np.stack(
        [np.ones(T, dtype=np.float32), np.arange(T, dtype=np.float32)]
    )
    core_ids = list(range(NCORES))
    in_maps = [
        {"vals": values[c * BP:(c + 1) * BP], "consts": consts}
        for c in core_ids
    ]

    trace = bool(int(os.environ.get("BASS_KERNEL_TRACE", "0")))
    res = run_bass_kernel_spmd(nc, in_maps, core_ids, trace=trace)
    LAST["exec_time_ns"] = res.exec_time_ns
    LAST["results"] = res

    context = np.concatenate([res.results[c]["ctx_out"] for c in core_ids], axis=0)
    attn = np.concatenate([res.results[c]["attn_out"] for c in core_ids], axis=0)
    coverage = np.concatenate([res.results[c]["cov_out"] for c in core_ids], axis=0)
    return context, attn, coverage


# revision 53
# speedup vs baseline: 1.1457x; 1.0178x over previous
"""Bahdanau-attention kernel for Trainium2 (8 NeuronCores, SPMD data parallel).

Math: the reference's per-step softmax is over a singleton axis, so the
attention weights are exactly 1.0. Hence:
    context  = values.sum(axis=1)            [B, DV]
    attn     = ones(B, T, 1)
    coverage[b, t, 0] = t                    [B, T, 1]
The W1/W2/W3/V MLP cancels out of every output.

Device work: per core, reduce a [B/8, T, DV] shard of `values` over T.
All 64 chunk loads stream on the sync HWDGE ring (outputs go on the
scalar ring so they never head-of-line block the loads); the fp32 adds
are spread over three engines so none exceeds the HBM/DMA roofline:
  - DVE: two short tensor_add chains per batch (dacc_a, dacc_b)
  - GpSimd: 4 chunks per batch in its own chain (gacc), folded into
    dacc_a mid-stream
  - PE: contracts dacc_a/dacc_b over partitions vs a ones column into
    the per-batch PSUM accumulation group.
attn/coverage come from a tiny host const tensor, written out by DMA.
"""

import os
import numpy as np

B, T, DV = 32, 2048, 1024
NCORES = 8
BP = B // NCORES          # 4 batches per core
TCH = 128                 # t-chunk rows = SBUF partitions
NCH = T // TCH            # 16 chunks of [128, DV] per batch
NSPLIT = 512              # PSUM bank free-dim limit (f32)
NJ = DV // NSPLIT         # 2 psum column groups

# Engine roles (measured per-chunk costs: DVE add 1.22us, gpsimd add
# ~2.4us, PE direct contraction ~2.5us = 4 MM insts; gpsimd and DVE adds
# contend on SBUF ports, so gpsimd's share is kept small):
#   - gpsimd: 4 chunks per batch in its own chain (gacc)
#   - DVE: two short chains per batch (dacc_a over A_CHUNKS, dacc_b over
#     B_CHUNKS); gacc folds into dacc_a after chunk GP_FOLD_AFTER
#   - PE: only the two cross-partition contractions per batch
GP_CHUNKS = (1, 4, 7, 10)
A_CHUNKS = (0, 2, 3, 5, 6)
B_CHUNKS = (8, 9, 11, 12, 13, 14, 15)
PE_CHUNK = None
PE_CHUNK_LAST_B = None    # optional: last batch's chunk 15 direct on PE
GP_FOLD_AFTER = 13        # gacc -> dacc_a fold point (gacc done by ~k13)
PAIR_LOADS = True         # load chunk pairs as single 1MB DMAs
NPAIR = NCH // 2          # 8 pair-loads per batch
CROSSFADE = 2             # pair-loads interleaved across each batch boundary


def _load_schedule():
    """Pair-load order: batch-major, but each boundary crossfades the last
    CROSSFADE pair-loads of b with the first CROSSFADE of b+1, so the
    batch-end reduction convoy (gpsimd tail + fold + contraction) drains
    while fresh chunks with idle consumers keep freeing buffer slots."""
    order = []
    for b in range(BP):
        start = CROSSFADE if b > 0 else 0
        end = NPAIR - CROSSFADE if b < BP - 1 else NPAIR
        order.extend((b, p) for p in range(start, end))
        if b < BP - 1:
            for i in range(CROSSFADE):
                order.append((b, NPAIR - CROSSFADE + i))
                order.append((b + 1, i))
    assert sorted(order) == [(b, p) for b in range(BP) for p in range(NPAIR)]
    return order

_CACHE = {}
LAST = {}                 # exec_time_ns etc. for the test harness


def _build_nc():
    import concourse.tile as tile
    from concourse import bacc, mybir
    from contextlib import ExitStack

    f32 = mybir.dt.float32
    nc = bacc.Bacc(
        "TRN2", target_bir_lowering=False, debug=False, num_devices=NCORES
    )

    vals = nc.dram_tensor("vals", [BP, T, DV], f32, kind="ExternalInput").ap()
    consts = nc.dram_tensor("consts", [2, T], f32, kind="ExternalInput").ap()
    ctx_out = nc.dram_tensor("ctx_out", [BP, DV], f32, kind="ExternalOutput").ap()
    attn_out = nc.dram_tensor("attn_out", [BP, T, 1], f32, kind="ExternalOutput").ap()
    cov_out = nc.dram_tensor("cov_out", [BP, T, 1], f32, kind="ExternalOutput").ap()

    with tile.TileContext(nc) as tc, ExitStack() as ctx:
        cpool = ctx.enter_context(tc.tile_pool(name="const", bufs=1))
        vpool = ctx.enter_context(tc.tile_pool(name="vals", bufs=12))
        dpool = ctx.enter_context(tc.tile_pool(name="dacc", bufs=1))
        ppool = ctx.enter_context(tc.tile_pool(name="ps", bufs=1, space="PSUM"))
        opool = ctx.enter_context(tc.tile_pool(name="out", bufs=2))

        ones_t = cpool.tile([128, 1], f32)
        nc.vector.memset(ones_t[:], 1.0)

        const_t = cpool.tile([2, T], f32)
        nc.sync.dma_start(out=const_t[:], in_=consts[:])

        # attn/coverage writes go on the scalar HWDGE ring, issued as soon
        # as const_t lands, so the sync ring carries only the big loads.
        for b in range(BP):
            nc.scalar.dma_start(out=attn_out[b:b + 1, :, 0], in_=const_t[0:1, :])
            nc.scalar.dma_start(out=cov_out[b:b + 1, :, 0], in_=const_t[1:2, :])

        st = {}
        for b in range(BP):
            st[b] = {
                "dacc_a": dpool.tile(
                    [TCH, DV], f32, name=f"dacca{b}", tag=f"dacca{b}"),
                "dacc_b": dpool.tile(
                    [TCH, DV], f32, name=f"daccb{b}", tag=f"daccb{b}"),
                "gacc": dpool.tile(
                    [TCH, DV], f32, name=f"gacc{b}", tag=f"gacc{b}"),
                "na": 0, "nb": 0, "ngp": 0,
                "afirst": None, "bfirst": None, "gfirst": None,
                "pe_tile": None,
            }

        def dispatch(s, b, k, vt):
            """Feed one [128, DV] chunk view to its assigned engine. The
            last batch skips gpsimd entirely (its chain + fold would land
            on the kernel tail, where gpsimd/DVE port contention bites) --
            its GP chunks join the DVE chains by k<8 instead."""
            last_b = b == BP - 1
            if k == PE_CHUNK or (last_b and k == PE_CHUNK_LAST_B):
                s["pe_tile"] = vt
            elif k in GP_CHUNKS and not last_b:
                s["ngp"] += 1
                if s["ngp"] == 1:
                    s["gfirst"] = vt
                elif s["ngp"] == 2:
                    nc.gpsimd.tensor_add(s["gacc"][:], s["gfirst"], vt)
                else:
                    nc.gpsimd.tensor_add(s["gacc"][:], s["gacc"][:], vt)
            elif k in A_CHUNKS or (last_b and k in GP_CHUNKS and k < 8):
                s["na"] += 1
                if s["na"] == 1:
                    s["afirst"] = vt
                elif s["na"] == 2:
                    nc.vector.tensor_add(s["dacc_a"][:], s["afirst"], vt)
                else:
                    nc.vector.tensor_add(s["dacc_a"][:], s["dacc_a"][:], vt)
            else:
                s["nb"] += 1
                if s["nb"] == 1:
                    s["bfirst"] = vt
                elif s["nb"] == 2:
                    nc.vector.tensor_add(s["dacc_b"][:], s["bfirst"], vt)
                else:
                    nc.vector.tensor_add(s["dacc_b"][:], s["dacc_b"][:], vt)
            if k == GP_FOLD_AFTER and not last_b:
                nc.vector.tensor_add(s["dacc_a"][:], s["dacc_a"][:], s["gacc"][:])

        for b, p in _load_schedule():
            s = st[b]
            # one 1MB DMA covers chunks (2p, 2p+1): [p, g, d] view of vals
            vr = vals[b].rearrange("(g p) d -> p g d", p=TCH)
            pt = vpool.tile([TCH, 2, DV], f32, name=f"pt{b}_{p}", tag="pt")
            nc.sync.dma_start(out=pt[:], in_=vr[:, 2 * p:2 * p + 2, :])
            dispatch(s, b, 2 * p, pt[:, 0, :])
            dispatch(s, b, 2 * p + 1, pt[:, 1, :])

            if p != NPAIR - 1:
                continue
            # b's final chunk scheduled: contract and write out
            ps = [
                ppool.tile([1, NSPLIT], f32, name=f"ps{b}_{j}", tag=f"ps{b}_{j}")
                for j in range(NJ)
            ]
            srcs = [s["dacc_a"], s["dacc_b"]]
            if s["pe_tile"] is not None:
                srcs.append(s["pe_tile"])
            for i, src in enumerate(srcs):
                for j in range(NJ):
                    sl = slice(j * NSPLIT, (j + 1) * NSPLIT)
                    nc.tensor.matmul(
                        ps[j][:], ones_t[:], src[:, sl],
                        start=(i == 0), stop=(i == len(srcs) - 1))

            ot = opool.tile([1, DV], f32, name=f"ot{b}", tag="ot")
            for j in range(NJ):
                nc.scalar.copy(ot[:, j * NSPLIT:(j + 1) * NSPLIT], ps[j][:])
            # context write on the scalar ring: keeps the sync HWDGE FIFO
            # free of output DMAs that would head-of-line block later loads
            nc.scalar.dma_start(out=ctx_out[b:b + 1, :], in_=ot[0:1, :])

    nc.compile()
    return nc


def kernel(query=None, values=None, **unused_weights):
    from concourse.bass_utils import run_bass_kernel_spmd

    values = np.ascontiguousarray(np.asarray(values, dtype=np.float32))
    assert values.shape == (B, T, DV), values.shape

    if "nc" not in _CACHE:
        _CACHE["nc"] = _build_nc()
    nc = _CACHE["nc"]

    consts = np.stack(
        [np.ones(T, dtype=np.float32), np.arange(T, dtype=np.float32)]
    )
    core_ids = list(range(NCORES))
    in_maps = [
        {"vals": values[c * BP:(c + 1) * BP], "consts": consts}
        for c in core_ids
    ]

    trace = bool(int(os.environ.get("BASS_KERNEL_TRACE", "0")))
    res = run_bass_kernel_spmd(nc, in_maps, core_ids, trace=trace)
    LAST["exec_time_ns"] = res.exec_time_ns
    LAST["results"] = res

    context = np.concatenate([res.results[c]["ctx_out"] for c in core_ids], axis=0)
    attn = np.concatenate([res.results[c]["attn_out"] for c in core_ids], axis=0)
    coverage = np.concatenate([res.results[c]["cov_out"] for c in core_ids], axis=0)
    return context, attn, coverage


# revision 58
# speedup vs baseline: 1.1682x; 1.0196x over previous
"""Bahdanau-attention kernel for Trainium2 (8 NeuronCores, SPMD data parallel).

Math: the reference's per-step softmax is over a singleton axis, so the
attention weights are exactly 1.0. Hence:
    context  = values.sum(axis=1)            [B, DV]
    attn     = ones(B, T, 1)
    coverage[b, t, 0] = t                    [B, T, 1]
The W1/W2/W3/V MLP cancels out of every output.

Device work: per core, reduce a [B/8, T, DV] shard of `values` over T.
32 x 1MB pair-loads stream on the sync HWDGE ring, crossfaded across
batch boundaries so batch-end reduction convoys never stall the loads
(outputs go on the scalar ring so they cannot head-of-line block them);
the fp32 adds are spread over three engines so none exceeds the HBM/DMA
roofline:
  - DVE: two short tensor_add chains per batch (dacc_a, dacc_b)
  - GpSimd: 4 chunks per batch in its own chain (gacc), folded into
    dacc_a mid-stream -- except the last batch, which skips gpsimd so
    the kernel tail has no gpsimd/DVE port contention
  - PE: contracts dacc_a/dacc_b over partitions vs a ones column into
    the per-batch PSUM accumulation group.
attn/coverage come from a tiny host const tensor, written out by DMA.
"""

import os
import numpy as np

B, T, DV = 32, 2048, 1024
NCORES = 8
BP = B // NCORES          # 4 batches per core
TCH = 128                 # t-chunk rows = SBUF partitions
NCH = T // TCH            # 16 chunks of [128, DV] per batch
NSPLIT = 512              # PSUM bank free-dim limit (f32)
NJ = DV // NSPLIT         # 2 psum column groups

# Engine roles (measured per-chunk costs: DVE add 1.22us, gpsimd add
# ~2.4us, PE direct contraction ~2.5us = 4 MM insts; gpsimd and DVE adds
# contend on SBUF ports, so gpsimd's share is kept small):
#   - gpsimd: 4 chunks per batch in its own chain (gacc)
#   - DVE: two short chains per batch (dacc_a over A_CHUNKS, dacc_b over
#     B_CHUNKS); gacc folds into dacc_a after chunk GP_FOLD_AFTER
#   - PE: only the two cross-partition contractions per batch
GP_CHUNKS = (1, 4, 7, 10)
A_CHUNKS = (0, 2, 3, 5, 6)
B_CHUNKS = (8, 9, 11, 12, 13, 14, 15)
PE_CHUNK = None
# last batch: late chunks contract directly on PE (idle at stream end)
# instead of queueing behind DVE's add backlog; 15 is the group stop
PE_LAST_B = (11, 13, 15)
GP_FOLD_AFTER = 13        # gacc -> dacc_a fold point (gacc done by ~k13)
PAIR_LOADS = True         # load chunk pairs as single 1MB DMAs
NPAIR = NCH // 2          # 8 pair-loads per batch
CROSSFADE = 2             # pair-loads interleaved across each batch boundary


def _load_schedule():
    """Pair-load order: batch-major, but each boundary crossfades the last
    CROSSFADE pair-loads of b with the first CROSSFADE of b+1, so the
    batch-end reduction convoy (gpsimd tail + fold + contraction) drains
    while fresh chunks with idle consumers keep freeing buffer slots."""
    order = []
    for b in range(BP):
        start = CROSSFADE if b > 0 else 0
        end = NPAIR - CROSSFADE if b < BP - 1 else NPAIR
        order.extend((b, p) for p in range(start, end))
        if b < BP - 1:
            for i in range(CROSSFADE):
                order.append((b, NPAIR - CROSSFADE + i))
                order.append((b + 1, i))
    assert sorted(order) == [(b, p) for b in range(BP) for p in range(NPAIR)]
    return order

_CACHE = {}
LAST = {}                 # exec_time_ns etc. for the test harness


def _build_nc():
    import concourse.tile as tile
    from concourse import bacc, mybir
    from contextlib import ExitStack

    f32 = mybir.dt.float32
    nc = bacc.Bacc(
        "TRN2", target_bir_lowering=False, debug=False, num_devices=NCORES
    )

    vals = nc.dram_tensor("vals", [BP, T, DV], f32, kind="ExternalInput").ap()
    consts = nc.dram_tensor("consts", [2, T], f32, kind="ExternalInput").ap()
    ctx_out = nc.dram_tensor("ctx_out", [BP, DV], f32, kind="ExternalOutput").ap()
    attn_out = nc.dram_tensor("attn_out", [BP, T, 1], f32, kind="ExternalOutput").ap()
    cov_out = nc.dram_tensor("cov_out", [BP, T, 1], f32, kind="ExternalOutput").ap()

    with tile.TileContext(nc) as tc, ExitStack() as ctx:
        cpool = ctx.enter_context(tc.tile_pool(name="const", bufs=1))
        vpool = ctx.enter_context(tc.tile_pool(name="vals", bufs=12))
        dpool = ctx.enter_context(tc.tile_pool(name="dacc", bufs=1))
        ppool = ctx.enter_context(tc.tile_pool(name="ps", bufs=1, space="PSUM"))
        opool = ctx.enter_context(tc.tile_pool(name="out", bufs=2))

        ones_t = cpool.tile([128, 1], f32)
        nc.vector.memset(ones_t[:], 1.0)

        const_t = cpool.tile([2, T], f32)
        nc.sync.dma_start(out=const_t[:], in_=consts[:])

        # attn/coverage writes go on the scalar HWDGE ring, issued as soon
        # as const_t lands, so the sync ring carries only the big loads.
        for b in range(BP):
            nc.scalar.dma_start(out=attn_out[b:b + 1, :, 0], in_=const_t[0:1, :])
            nc.scalar.dma_start(out=cov_out[b:b + 1, :, 0], in_=const_t[1:2, :])

        st = {}
        for b in range(BP):
            st[b] = {
                "dacc_a": dpool.tile(
                    [TCH, DV], f32, name=f"dacca{b}", tag=f"dacca{b}"),
                "dacc_b": dpool.tile(
                    [TCH, DV], f32, name=f"daccb{b}", tag=f"daccb{b}"),
                "gacc": dpool.tile(
                    [TCH, DV], f32, name=f"gacc{b}", tag=f"gacc{b}"),
                "na": 0, "nb": 0, "ngp": 0,
                "afirst": None, "bfirst": None, "gfirst": None,
                "pe_tiles": {},
            }

        def dispatch(s, b, k, vt):
            """Feed one [128, DV] chunk view to its assigned engine. The
            last batch skips gpsimd entirely (its chain + fold would land
            on the kernel tail, where gpsimd/DVE port contention bites) --
            its GP chunks join the DVE chains by k<8 instead."""
            last_b = b == BP - 1
            if k == PE_CHUNK or (last_b and k in PE_LAST_B):
                s["pe_tiles"][k] = vt
            elif k in GP_CHUNKS and not last_b:
                s["ngp"] += 1
                if s["ngp"] == 1:
                    s["gfirst"] = vt
                elif s["ngp"] == 2:
                    nc.gpsimd.tensor_add(s["gacc"][:], s["gfirst"], vt)
                else:
                    nc.gpsimd.tensor_add(s["gacc"][:], s["gacc"][:], vt)
            elif k in A_CHUNKS or (last_b and k in GP_CHUNKS and k < 8):
                s["na"] += 1
                if s["na"] == 1:
                    s["afirst"] = vt
                elif s["na"] == 2:
                    nc.vector.tensor_add(s["dacc_a"][:], s["afirst"], vt)
                else:
                    nc.vector.tensor_add(s["dacc_a"][:], s["dacc_a"][:], vt)
            else:
                s["nb"] += 1
                if s["nb"] == 1:
                    s["bfirst"] = vt
                elif s["nb"] == 2:
                    nc.vector.tensor_add(s["dacc_b"][:], s["bfirst"], vt)
                else:
                    nc.vector.tensor_add(s["dacc_b"][:], s["dacc_b"][:], vt)
            if k == GP_FOLD_AFTER and not last_b:
                nc.vector.tensor_add(s["dacc_a"][:], s["dacc_a"][:], s["gacc"][:])

        for b, p in _load_schedule():
            s = st[b]
            # one 1MB DMA covers chunks (2p, 2p+1): [p, g, d] view of vals
            vr = vals[b].rearrange("(g p) d -> p g d", p=TCH)
            pt = vpool.tile([TCH, 2, DV], f32, name=f"pt{b}_{p}", tag="pt")
            nc.sync.dma_start(out=pt[:], in_=vr[:, 2 * p:2 * p + 2, :])
            dispatch(s, b, 2 * p, pt[:, 0, :])
            dispatch(s, b, 2 * p + 1, pt[:, 1, :])

            if p != NPAIR - 1:
                continue
            # b's final chunk scheduled: contract and write out
            ps = [
                ppool.tile([1, NSPLIT], f32, name=f"ps{b}_{j}", tag=f"ps{b}_{j}")
                for j in range(NJ)
            ]
            # readiness order: dacc_a (mid-batch), early direct chunks,
            # dacc_b (one add after its last chunk), last direct chunk stops
            pk = sorted(s["pe_tiles"])
            srcs = [s["dacc_a"]]
            srcs += [s["pe_tiles"][k] for k in pk[:-1]]
            srcs.append(s["dacc_b"])
            srcs += [s["pe_tiles"][k] for k in pk[-1:]]
            for i, src in enumerate(srcs):
                for j in range(NJ):
                    sl = slice(j * NSPLIT, (j + 1) * NSPLIT)
                    nc.tensor.matmul(
                        ps[j][:], ones_t[:], src[:, sl],
                        start=(i == 0), stop=(i == len(srcs) - 1))

            ot = opool.tile([1, DV], f32, name=f"ot{b}", tag="ot")
            for j in range(NJ):
                nc.scalar.copy(ot[:, j * NSPLIT:(j + 1) * NSPLIT], ps[j][:])
            # context write on the scalar ring: keeps the sync HWDGE FIFO
            # free of output DMAs that would head-of-line block later loads
            nc.scalar.dma_start(out=ctx_out[b:b + 1, :], in_=ot[0:1, :])

    nc.compile()
    return nc


def kernel(query=None, values=None, **unused_weights):
    from concourse.bass_utils import run_bass_kernel_spmd

    values = np.ascontiguousarray(np.asarray(values, dtype=np.float32))
    assert values.shape == (B, T, DV), values.shape

    if "nc" not in _CACHE:
        _CACHE["nc"] = _build_nc()
    nc = _CACHE["nc"]

    consts = np.stack(
        [np.ones(T, dtype=np.float32), np.arange(T, dtype=np.float32)]
    )
    core_ids = list(range(NCORES))
    in_maps = [
        {"vals": values[c * BP:(c + 1) * BP], "consts": consts}
        for c in core_ids
    ]

    trace = bool(int(os.environ.get("BASS_KERNEL_TRACE", "0")))
    res = run_bass_kernel_spmd(nc, in_maps, core_ids, trace=trace)
    LAST["exec_time_ns"] = res.exec_time_ns
    LAST["results"] = res

    context = np.concatenate([res.results[c]["ctx_out"] for c in core_ids], axis=0)
    attn = np.concatenate([res.results[c]["attn_out"] for c in core_ids], axis=0)
    coverage = np.concatenate([res.results[c]["cov_out"] for c in core_ids], axis=0)
    return context, attn, coverage
